# revision 22
# baseline (speedup 1.0000x reference)
"""Distributed Trainium2 kernel for varlen GQA prefill attention with a
paged-KV-cache scatter (vLLM-style store_kvcache + flash_attn_varlen).

Sharding (8 NeuronCores): tensor-parallel over the 4 KV heads (4 groups
x 4 query heads each) x data-parallel over the 2 token halves (the 4
sequences of 512 tokens split 2/2). Each core's output slice is
disjoint, so no collectives are needed; the KV-cache scatter/gather is
replicated per shard on that shard's kv-head slice.
"""

import sys

for _p in ("/opt/trn_rl_repo", "/opt/trn_rl_repo/concourse"):
    if _p not in sys.path:
        sys.path.insert(0, _p)

import math

import ml_dtypes
import numpy as np

import concourse.bass as bass
import concourse.mybir as mybir
import concourse.tile as tile
from concourse import bacc
from concourse.bass import ds, ts
from concourse.bass_utils import run_bass_kernel_spmd
from concourse.masks import make_identity

BF16 = ml_dtypes.bfloat16

N = 2048
HQ = 16
HKV = 4
D = 128
NUM_SLOTS = 131072
SEQ = 512
SCALE = 1.0 / math.sqrt(D)

P = 128
N_CORES = 8
TOK = N // 2          # tokens per core (two halves)
NSEG = TOK // SEQ     # segments per core (2)
NH = HQ // HKV        # q heads per core (4)
NT = TOK // P         # 128-token tiles per core (8)
NKT = SEQ // P        # 128-token tiles per segment (4)

_nc_cache = {}


def build(honest: bool, variant: str = "full"):
    nc = bacc.Bacc(None, target_bir_lowering=False)
    f32 = mybir.dt.float32
    bf16 = mybir.dt.bfloat16
    i32 = mybir.dt.int32

    qT_in = nc.declare_dram_parameter("qT", [P, NH, TOK], bf16, isOutput=False)
    tri_in = nc.declare_dram_parameter("tri", [P, NKT, P], bf16, isOutput=False)
    if honest:
        kvR_in = nc.declare_dram_parameter("kvR", [P, NT, 2 * D], bf16, isOutput=False)
        sl_in = nc.declare_dram_parameter("slots", [P, NT], i32, isOutput=False)
    if (not honest) or variant == "vA_rhs":
        kT_in = nc.declare_dram_parameter("kT", [P, TOK], bf16, isOutput=False)
        vA_in = nc.declare_dram_parameter("vA", [P, NT, D + 1], bf16, isOutput=False)
    o_out = nc.declare_dram_parameter("o", [P, NH, NT, D], bf16, isOutput=True)

    with tile.TileContext(nc) as tc:
        with (
            tc.tile_pool(name="persist", bufs=1) as pp,
            tc.tile_pool(name="sc_psum", bufs=2, space="PSUM") as scp,
            tc.tile_pool(name="pv_psum", bufs=2, space="PSUM") as pvp,
            tc.tile_pool(name="warm_psum", bufs=1, space="PSUM") as wmp,
            tc.tile_pool(name="work", bufs=2) as wp,
            tc.tile_pool(name="small", bufs=4) as sp,
        ):
            qT_sb = pp.tile([P, NH, TOK], bf16, tag="qT_sb")
            tri_sb = pp.tile([P, NKT, P], bf16, tag="tri_sb")
            kT_sb = pp.tile([P, TOK], bf16, tag="kT_sb")
            vA_sb = pp.tile([P, NT, D + 1], bf16, tag="vA_sb")
            o_sb = pp.tile([P, NH, NT, D], bf16, tag="o_sb")

            # Warm up the PE HAM clock-gate while the input DMAs land:
            # dummy matmuls on a scratch tile keep TensorE busy >3.4us so
            # the real matmuls run at 2.4GHz from the start.
            junk_sb = pp.tile([P, SEQ], bf16, tag="junk_sb")
            junk_ps = wmp.tile([P, SEQ], f32, tag="junk_ps")
            nc.gpsimd.memset(junk_sb[:], 0.125)
            for _ in range(14):
                nc.tensor.matmul(
                    junk_ps[:], lhsT=junk_sb[:, 0:P], rhs=junk_sb[:],
                    start=True, stop=True,
                )

            if not honest:
                nc.sync.dma_start(out=kT_sb[:], in_=kT_in[:])
            for h in range(NH):
                nc.sync.dma_start(out=qT_sb[:, h, :], in_=qT_in[:, h, :])
            if (not honest) or variant == "vA_rhs":
                nc.sync.dma_start(out=vA_sb[:], in_=vA_in[:])
            nc.sync.dma_start(out=tri_sb[:], in_=tri_in[:])

            if honest and variant != "attn_only":
                with tc.tile_pool(name="tables", bufs=1, space="DRAM") as dp, \
                     tc.tile_pool(name="tp_psum", bufs=2, space="PSUM") as tpp:
                    # one private [NUM_SLOTS, 256] kv table per 128-token
                    # tile so the 8 scatter->gather pairs stay independent
                    tables = [
                        dp.tile([NUM_SLOTS, 2 * D], bf16, name=f"kv_table{c}", tag=f"kv_table{c}")
                        for c in range(NT)
                    ]
                    kvR_sb = pp.tile([P, NT, 2 * D], bf16, tag="kvR_sb")
                    kvG_sb = pp.tile([P, NT, 2 * D + 2], bf16, tag="kvG_sb")
                    sl_sb = pp.tile([P, NT], i32, tag="sl_sb")
                    ident = pp.tile([P, P], bf16, tag="ident")
                    make_identity(nc, ident[:])

                    nc.sync.dma_start(out=sl_sb[:], in_=sl_in[:])
                    nc.sync.dma_start(out=kvR_sb[:], in_=kvR_in[:])
                    nc.vector.memset(kvG_sb[:, :, 2 * D : 2 * D + 1], 1.0)

                    for c in range(NT):
                        # scatter the 128 [k|v] rows of tile c, read them
                        # back (the paged-read), transpose K for the QK^T
                        nc.gpsimd.indirect_dma_start(
                            out=tables[c][:],
                            out_offset=bass.IndirectOffsetOnAxis(
                                ap=sl_sb[:, c : c + 1], axis=0
                            ),
                            in_=kvR_sb[:, c, :],
                            in_offset=None,
                        )
                        nc.gpsimd.indirect_dma_start(
                            out=kvG_sb[:, c, 0 : 2 * D],
                            out_offset=None,
                            in_=tables[c][:],
                            in_offset=bass.IndirectOffsetOnAxis(
                                ap=sl_sb[:, c : c + 1], axis=0
                            ),
                        )
                        tp = tpp.tile([P, P], bf16, tag="tp")
                        nc.tensor.transpose(tp[:], kvG_sb[:, c, 0:D], ident[:])
                        nc.vector.tensor_copy(out=kT_sb[:, ts(c, P)], in_=tp[:])
            if variant == "scatter_only":
                nc.vector.memset(o_sb[:], 0.0)
                nc.sync.dma_start(out=o_out[:], in_=o_sb[:])
            for seg in range(NSEG if variant != "scatter_only" else 0):
                for h in range(NH):
                    expT = wp.tile([P, NKT, SEQ], bf16, tag="expT")
                    for kp in range(NKT // 2):
                        kt0 = 2 * kp
                        n_q0 = SEQ - kt0 * P
                        n_q1 = SEQ - (kt0 + 1) * P
                        sc = scp.tile([P, 2, SEQ], f32, tag="sc")
                        for j, (kt, n_q) in enumerate(
                            ((kt0, n_q0), (kt0 + 1, n_q1))
                        ):
                            q0 = seg * SEQ + kt * P
                            nc.tensor.matmul(
                                sc[:, j, :n_q],
                                lhsT=kT_sb[:, ds(seg * SEQ + kt * P, P)],
                                rhs=qT_sb[:, h, ds(q0, n_q)],
                                start=True,
                                stop=True,
                            )
                        # one exp over both banks; the tail of block kt0+1
                        # past n_q1 is never-read scratch
                        nc.scalar.activation(
                            expT[:, kt0 : kt0 + 2, :n_q0],
                            sc[:, :, :n_q0],
                            mybir.ActivationFunctionType.Exp,
                            scale=SCALE,
                        )
                    # mask the (upper-triangular in [k,q]) diagonal blocks;
                    # runs on the otherwise-idle GpSimd engine
                    nc.gpsimd.tensor_tensor(
                        out=expT[:, :, 0:P],
                        in0=expT[:, :, 0:P],
                        in1=tri_sb[:],
                        op=mybir.AluOpType.mult,
                    )
                    for qt in range(NKT):
                        pv = pvp.tile([P, D + 1], f32, tag="pv")
                        for kt in range(qt + 1):
                            c = seg * NKT + kt
                            if honest and variant != "vA_rhs":
                                rhs = kvG_sb[:, c, D : 2 * D + 1]
                            else:
                                rhs = vA_sb[:, c, :]
                            nc.tensor.matmul(
                                pv[:],
                                lhsT=expT[:, kt, ds((qt - kt) * P, P)],
                                rhs=rhs,
                                start=(kt == 0),
                                stop=(kt == qt),
                            )
                        rec = sp.tile([P, 1], f32, tag="rec")
                        nc.vector.reciprocal(rec[:], pv[:, D : D + 1])
                        nc.vector.tensor_scalar_mul(
                            o_sb[:, h, seg * NKT + qt, :], pv[:, 0:D], rec[:, 0:1]
                        )
                    nc.sync.dma_start(
                        out=o_out[:, h, ds(seg * NKT, NKT), :],
                        in_=o_sb[:, h, ds(seg * NKT, NKT), :],
                    )
    nc.compile()
    return nc


def _shard_inputs(q, k, v, slot_mapping):
    tri = (np.arange(P)[:, None] <= np.arange(P)[None, :]).astype(BF16)
    tri = np.ascontiguousarray(np.broadcast_to(tri[:, None, :], (P, NKT, P)))
    in_maps = []
    for c in range(N_CORES):
        hg, tg = c // 2, c % 2
        t0 = tg * TOK
        q_sh = q[t0 : t0 + TOK, hg * NH : (hg + 1) * NH, :]
        qT = np.ascontiguousarray(q_sh.transpose(2, 1, 0)).astype(BF16)
        k_sh = k[t0 : t0 + TOK, hg, :]
        v_sh = v[t0 : t0 + TOK, hg, :]
        kvR = np.empty((P, NT, 2 * D), dtype=BF16)
        kvR[:, :, :D] = k_sh.reshape(NT, P, P).transpose(1, 0, 2)
        kvR[:, :, D:] = v_sh.reshape(NT, P, P).transpose(1, 0, 2)
        kT = np.ascontiguousarray(k_sh.T).astype(BF16)
        vA = np.empty((P, NT, D + 1), dtype=BF16)
        vA[:, :, :D] = kvR[:, :, D:]
        vA[:, :, D] = 1.0
        slots = np.ascontiguousarray(
            slot_mapping[t0 : t0 + TOK].reshape(NT, P).T
        ).astype(np.int32)
        in_maps.append(
            {
                "qT": qT,
                "tri": tri,
                "kT": kT,
                "vA": vA,
                "kvR": kvR,
                "slots": slots,
            }
        )
    return in_maps


def _assemble(results):
    out = np.empty((N, HQ, D), dtype=np.float32)
    for c in range(N_CORES):
        hg, tg = c // 2, c % 2
        t0 = tg * TOK
        oc = np.asarray(results[c]["o"]).astype(np.float32)  # [P, NH, NT, D]
        # token t0 + ct*128 + p, head hg*NH + h  <-  oc[p, h, ct, :]
        out[t0 : t0 + TOK, hg * NH : (hg + 1) * NH, :] = oc.transpose(
            2, 0, 1, 3
        ).reshape(TOK, NH, D)
    return out


def _numpy_reference(q, k, v, k_cache, v_cache, slot_mapping, cu_seqlens):
    """Bit-faithful numpy fallback used only if inputs don't match the
    shapes/metadata this kernel was specialized for."""
    n = q.shape[0]
    k_cache = np.array(k_cache, dtype=np.float32, copy=True)
    v_cache = np.array(v_cache, dtype=np.float32, copy=True)
    sm = slot_mapping.astype(np.int64)
    valid = sm >= 0
    k_cache[sm[valid]] = k.reshape(n, -1)[valid]
    v_cache[sm[valid]] = v.reshape(n, -1)[valid]
    read = np.clip(sm, 0, k_cache.shape[0] - 1)
    kc = k_cache[read].reshape(n, HKV, D)
    vc = v_cache[read].reshape(n, HKV, D)
    pos = np.arange(n)
    seg = np.searchsorted(cu_seqlens, pos, side="right") - 1
    group = q.shape[1] // kc.shape[1]
    ke = np.repeat(kc, group, axis=1)
    ve = np.repeat(vc, group, axis=1)
    scores = np.einsum("qhd,khd->hqk", q, ke, dtype=np.float32) * np.float32(SCALE)
    mask = (seg[:, None] == seg[None, :]) & (pos[None, :] <= pos[:, None])
    scores = np.where(mask[None], scores, -np.inf)
    scores -= scores.max(axis=-1, keepdims=True)
    p = np.exp(scores)
    p /= p.sum(axis=-1, keepdims=True)
    return np.einsum("hqk,khd->qhd", p, ve).astype(np.float32)


def _inputs_match_specialization(q, k, v, k_cache, v_cache, slot_mapping, cu_seqlens):
    if q.shape != (N, HQ, D) or k.shape != (N, HKV, D) or v.shape != (N, HKV, D):
        return False
    if k_cache.shape != (NUM_SLOTS, HKV * D) or v_cache.shape != (NUM_SLOTS, HKV * D):
        return False
    if not np.array_equal(cu_seqlens, np.arange(0, N + 1, SEQ)):
        return False
    sm = np.asarray(slot_mapping)
    if sm.shape != (N,):
        return False
    if sm.min() < 0 or sm.max() >= NUM_SLOTS:
        return False
    if np.unique(sm).size != N:
        return False
    # kernel assumes the caches start zeroed only insofar as unwritten
    # slots are never read back, which holds when all slots are distinct
    return True


def _get_nc(honest: bool, variant: str = "full"):
    key = ("honest" if honest else "skip", variant)
    if key not in _nc_cache:
        _nc_cache[key] = build(honest, variant)
    return _nc_cache[key]


HONEST = True
VARIANT = "full"


def kernel(q, k, v, k_cache, v_cache, slot_mapping, cu_seqlens, _trace=False):
    q = np.asarray(q, dtype=np.float32)
    k = np.asarray(k, dtype=np.float32)
    v = np.asarray(v, dtype=np.float32)
    slot_mapping = np.asarray(slot_mapping, dtype=np.int32)
    cu_seqlens = np.asarray(cu_seqlens, dtype=np.int32)

    if not _inputs_match_specialization(
        q, k, v, k_cache, v_cache, slot_mapping, cu_seqlens
    ):
        return _numpy_reference(
            q, k, v, k_cache, v_cache, slot_mapping, cu_seqlens
        )

    nc = _get_nc(HONEST, VARIANT)
    in_maps = _shard_inputs(q, k, v, slot_mapping)
    res = run_bass_kernel_spmd(
        nc, in_maps, core_ids=list(range(N_CORES)), trace=_trace
    )
    out = _assemble(res.results)
    if _trace:
        kernel._last_bench = res
    return out


# revision 23
# speedup vs baseline: 1.2096x; 1.2096x over previous
"""Distributed Trainium2 kernel for varlen GQA prefill attention with a
paged-KV-cache scatter (vLLM-style store_kvcache + flash_attn_varlen).

Sharding (8 NeuronCores): tensor-parallel over the 4 KV heads (4 groups
x 4 query heads each) x data-parallel over the 2 token halves (the 4
sequences of 512 tokens split 2/2). Each core's output slice is
disjoint, so no collectives are needed; the KV-cache scatter/gather is
replicated per shard on that shard's kv-head slice.
"""

import sys

for _p in ("/opt/trn_rl_repo", "/opt/trn_rl_repo/concourse"):
    if _p not in sys.path:
        sys.path.insert(0, _p)

import math

import ml_dtypes
import numpy as np

import concourse.bass as bass
import concourse.mybir as mybir
import concourse.tile as tile
from concourse import bacc
from concourse.bass import ds, ts
from concourse.bass_utils import run_bass_kernel_spmd
from concourse.masks import make_identity

BF16 = ml_dtypes.bfloat16

N = 2048
HQ = 16
HKV = 4
D = 128
NUM_SLOTS = 131072
SEQ = 512
SCALE = 1.0 / math.sqrt(D)

P = 128
N_CORES = 8
TOK = N // 2          # tokens per core (two halves)
NSEG = TOK // SEQ     # segments per core (2)
NH = HQ // HKV        # q heads per core (4)
NT = TOK // P         # 128-token tiles per core (8)
NKT = SEQ // P        # 128-token tiles per segment (4)

_nc_cache = {}


def build(honest: bool, variant: str = "full"):
    nc = bacc.Bacc(None, target_bir_lowering=False)
    f32 = mybir.dt.float32
    bf16 = mybir.dt.bfloat16
    i32 = mybir.dt.int32

    qT_in = nc.declare_dram_parameter("qT", [P, NH, TOK], bf16, isOutput=False)
    tri_in = nc.declare_dram_parameter("tri", [P, NKT, P], bf16, isOutput=False)
    if honest:
        kvR_in = nc.declare_dram_parameter("kvR", [P, NT, 2 * D], bf16, isOutput=False)
        sl_in = nc.declare_dram_parameter("slots", [P, NT], i32, isOutput=False)
    if (not honest) or variant == "vA_rhs":
        kT_in = nc.declare_dram_parameter("kT", [P, TOK], bf16, isOutput=False)
        vA_in = nc.declare_dram_parameter("vA", [P, NT, D + 1], bf16, isOutput=False)
    o_out = nc.declare_dram_parameter("o", [P, NH, NT, D], bf16, isOutput=True)

    with tile.TileContext(nc) as tc:
        with (
            tc.tile_pool(name="persist", bufs=1) as pp,
            tc.tile_pool(name="sc_psum", bufs=2, space="PSUM") as scp,
            tc.tile_pool(name="pv_psum", bufs=2, space="PSUM") as pvp,
            tc.tile_pool(name="warm_psum", bufs=1, space="PSUM") as wmp,
            tc.tile_pool(name="work", bufs=2) as wp,
            tc.tile_pool(name="small", bufs=4) as sp,
        ):
            qT_sb = pp.tile([P, NH, TOK], bf16, tag="qT_sb")
            tri_sb = pp.tile([P, NKT, P], bf16, tag="tri_sb")
            kT_sb = pp.tile([P, TOK], bf16, tag="kT_sb")
            vA_sb = pp.tile([P, NT, D + 1], bf16, tag="vA_sb")
            o_sb = pp.tile([P, NH, NT, D], bf16, tag="o_sb")

            # Warm up the PE HAM clock-gate while the input DMAs land:
            # dummy matmuls on a scratch tile keep TensorE busy >3.4us so
            # the real matmuls run at 2.4GHz from the start.
            junk_sb = pp.tile([P, SEQ], bf16, tag="junk_sb")
            junk_ps = wmp.tile([P, SEQ], f32, tag="junk_ps")
            nc.gpsimd.memset(junk_sb[:], 0.125)
            for _ in range(7):
                nc.tensor.matmul(
                    junk_ps[:], lhsT=junk_sb[:, 0:P], rhs=junk_sb[:],
                    start=True, stop=True,
                )

            if not honest:
                nc.sync.dma_start(out=kT_sb[:], in_=kT_in[:])
            for h in range(NH):
                nc.sync.dma_start(out=qT_sb[:, h, :], in_=qT_in[:, h, :])
            if (not honest) or variant == "vA_rhs":
                nc.sync.dma_start(out=vA_sb[:], in_=vA_in[:])
            nc.sync.dma_start(out=tri_sb[:], in_=tri_in[:])

            if honest and variant != "attn_only":
                with tc.tile_pool(name="tables", bufs=1, space="DRAM") as dp, \
                     tc.tile_pool(name="tp_psum", bufs=1, space="PSUM") as tpp:
                    # one private [NUM_SLOTS, 256] kv table per 128-token
                    # tile so the 8 scatter->gather pairs stay independent
                    tables = [
                        dp.tile([NUM_SLOTS, 2 * D], bf16, name=f"kv_table{c}", tag=f"kv_table{c}")
                        for c in range(NT)
                    ]
                    kvR_sb = pp.tile([P, NT, 2 * D], bf16, tag="kvR_sb")
                    kvG_sb = pp.tile([P, NT, 2 * D + 2], bf16, tag="kvG_sb")
                    sl_sb = pp.tile([P, NT], i32, tag="sl_sb")
                    ident = pp.tile([P, P], bf16, tag="ident")
                    make_identity(nc, ident[:])

                    nc.sync.dma_start(out=sl_sb[:], in_=sl_in[:])
                    nc.sync.dma_start(out=kvR_sb[:], in_=kvR_in[:])
                    nc.vector.memset(kvG_sb[:, :, 2 * D : 2 * D + 1], 1.0)

                    for c in range(NT):
                        # scatter the 128 [k|v] rows of tile c, read them
                        # back (the paged-read), transpose K for the QK^T
                        nc.gpsimd.indirect_dma_start(
                            out=tables[c][:],
                            out_offset=bass.IndirectOffsetOnAxis(
                                ap=sl_sb[:, c : c + 1], axis=0
                            ),
                            in_=kvR_sb[:, c, :],
                            in_offset=None,
                        )
                        nc.gpsimd.indirect_dma_start(
                            out=kvG_sb[:, c, 0 : 2 * D],
                            out_offset=None,
                            in_=tables[c][:],
                            in_offset=bass.IndirectOffsetOnAxis(
                                ap=sl_sb[:, c : c + 1], axis=0
                            ),
                        )
                        tp = tpp.tile([P, P], bf16, tag="tp")
                        nc.tensor.transpose(tp[:], kvG_sb[:, c, 0:D], ident[:])
                        nc.vector.tensor_copy(out=kT_sb[:, ts(c, P)], in_=tp[:])
            if variant == "scatter_only":
                nc.vector.memset(o_sb[:], 0.0)
                nc.sync.dma_start(out=o_out[:], in_=o_sb[:])
            for seg in range(NSEG if variant != "scatter_only" else 0):
                for h in range(NH):
                    expT = wp.tile([P, NKT, SEQ], bf16, tag="expT")
                    for kp in range(NKT // 2):
                        kt0 = 2 * kp
                        n_q0 = SEQ - kt0 * P
                        n_q1 = SEQ - (kt0 + 1) * P
                        sc = scp.tile([P, 2, SEQ], f32, tag="sc")
                        for j, (kt, n_q) in enumerate(
                            ((kt0, n_q0), (kt0 + 1, n_q1))
                        ):
                            q0 = seg * SEQ + kt * P
                            nc.tensor.matmul(
                                sc[:, j, :n_q],
                                lhsT=kT_sb[:, ds(seg * SEQ + kt * P, P)],
                                rhs=qT_sb[:, h, ds(q0, n_q)],
                                start=True,
                                stop=True,
                            )
                        # one exp over both banks; the tail of block kt0+1
                        # past n_q1 is never-read scratch
                        nc.scalar.activation(
                            expT[:, kt0 : kt0 + 2, :n_q0],
                            sc[:, :, :n_q0],
                            mybir.ActivationFunctionType.Exp,
                            scale=SCALE,
                        )
                        # mask the (upper-triangular in [k,q]) diagonal
                        # blocks on the otherwise-idle GpSimd engine
                        nc.gpsimd.tensor_tensor(
                            out=expT[:, kt0 : kt0 + 2, 0:P],
                            in0=expT[:, kt0 : kt0 + 2, 0:P],
                            in1=tri_sb[:, kt0 : kt0 + 2, :],
                            op=mybir.AluOpType.mult,
                        )
                    for qp in range(NKT // 2):
                        pv = pvp.tile([P, 2, D + 1], f32, tag="pv")
                        for j in range(2):
                            qt = 2 * qp + j
                            for kt in range(qt + 1):
                                c = seg * NKT + kt
                                if honest and variant != "vA_rhs":
                                    rhs = kvG_sb[:, c, D : 2 * D + 1]
                                else:
                                    rhs = vA_sb[:, c, :]
                                nc.tensor.matmul(
                                    pv[:, j, :],
                                    lhsT=expT[:, kt, ds((qt - kt) * P, P)],
                                    rhs=rhs,
                                    start=(kt == 0),
                                    stop=(kt == qt),
                                )
                        rec = sp.tile([P, 2], f32, tag="rec")
                        nc.vector.reciprocal(rec[:], pv[:, :, D])
                        nc.vector.tensor_tensor(
                            out=o_sb[:, h, ds(seg * NKT + 2 * qp, 2), :],
                            in0=pv[:, :, 0:D],
                            in1=rec[:, :, None].to_broadcast([P, 2, D]),
                            op=mybir.AluOpType.mult,
                        )
                    nc.sync.dma_start(
                        out=o_out[:, h, ds(seg * NKT, NKT), :],
                        in_=o_sb[:, h, ds(seg * NKT, NKT), :],
                    )
    nc.compile()
    return nc


def _shard_inputs(q, k, v, slot_mapping):
    tri = (np.arange(P)[:, None] <= np.arange(P)[None, :]).astype(BF16)
    tri = np.ascontiguousarray(np.broadcast_to(tri[:, None, :], (P, NKT, P)))
    in_maps = []
    for c in range(N_CORES):
        hg, tg = c // 2, c % 2
        t0 = tg * TOK
        q_sh = q[t0 : t0 + TOK, hg * NH : (hg + 1) * NH, :]
        qT = np.ascontiguousarray(q_sh.transpose(2, 1, 0)).astype(BF16)
        k_sh = k[t0 : t0 + TOK, hg, :]
        v_sh = v[t0 : t0 + TOK, hg, :]
        kvR = np.empty((P, NT, 2 * D), dtype=BF16)
        kvR[:, :, :D] = k_sh.reshape(NT, P, P).transpose(1, 0, 2)
        kvR[:, :, D:] = v_sh.reshape(NT, P, P).transpose(1, 0, 2)
        kT = np.ascontiguousarray(k_sh.T).astype(BF16)
        vA = np.empty((P, NT, D + 1), dtype=BF16)
        vA[:, :, :D] = kvR[:, :, D:]
        vA[:, :, D] = 1.0
        slots = np.ascontiguousarray(
            slot_mapping[t0 : t0 + TOK].reshape(NT, P).T
        ).astype(np.int32)
        in_maps.append(
            {
                "qT": qT,
                "tri": tri,
                "kT": kT,
                "vA": vA,
                "kvR": kvR,
                "slots": slots,
            }
        )
    return in_maps


def _assemble(results):
    out = np.empty((N, HQ, D), dtype=np.float32)
    for c in range(N_CORES):
        hg, tg = c // 2, c % 2
        t0 = tg * TOK
        oc = np.asarray(results[c]["o"]).astype(np.float32)  # [P, NH, NT, D]
        # token t0 + ct*128 + p, head hg*NH + h  <-  oc[p, h, ct, :]
        out[t0 : t0 + TOK, hg * NH : (hg + 1) * NH, :] = oc.transpose(
            2, 0, 1, 3
        ).reshape(TOK, NH, D)
    return out


def _numpy_reference(q, k, v, k_cache, v_cache, slot_mapping, cu_seqlens):
    """Bit-faithful numpy fallback used only if inputs don't match the
    shapes/metadata this kernel was specialized for."""
    n = q.shape[0]
    k_cache = np.array(k_cache, dtype=np.float32, copy=True)
    v_cache = np.array(v_cache, dtype=np.float32, copy=True)
    sm = slot_mapping.astype(np.int64)
    valid = sm >= 0
    k_cache[sm[valid]] = k.reshape(n, -1)[valid]
    v_cache[sm[valid]] = v.reshape(n, -1)[valid]
    read = np.clip(sm, 0, k_cache.shape[0] - 1)
    kc = k_cache[read].reshape(n, HKV, D)
    vc = v_cache[read].reshape(n, HKV, D)
    pos = np.arange(n)
    seg = np.searchsorted(cu_seqlens, pos, side="right") - 1
    group = q.shape[1] // kc.shape[1]
    ke = np.repeat(kc, group, axis=1)
    ve = np.repeat(vc, group, axis=1)
    scores = np.einsum("qhd,khd->hqk", q, ke, dtype=np.float32) * np.float32(SCALE)
    mask = (seg[:, None] == seg[None, :]) & (pos[None, :] <= pos[:, None])
    scores = np.where(mask[None], scores, -np.inf)
    scores -= scores.max(axis=-1, keepdims=True)
    p = np.exp(scores)
    p /= p.sum(axis=-1, keepdims=True)
    return np.einsum("hqk,khd->qhd", p, ve).astype(np.float32)


def _inputs_match_specialization(q, k, v, k_cache, v_cache, slot_mapping, cu_seqlens):
    if q.shape != (N, HQ, D) or k.shape != (N, HKV, D) or v.shape != (N, HKV, D):
        return False
    if k_cache.shape != (NUM_SLOTS, HKV * D) or v_cache.shape != (NUM_SLOTS, HKV * D):
        return False
    if not np.array_equal(cu_seqlens, np.arange(0, N + 1, SEQ)):
        return False
    sm = np.asarray(slot_mapping)
    if sm.shape != (N,):
        return False
    if sm.min() < 0 or sm.max() >= NUM_SLOTS:
        return False
    if np.unique(sm).size != N:
        return False
    # kernel assumes the caches start zeroed only insofar as unwritten
    # slots are never read back, which holds when all slots are distinct
    return True


def _get_nc(honest: bool, variant: str = "full"):
    key = ("honest" if honest else "skip", variant)
    if key not in _nc_cache:
        _nc_cache[key] = build(honest, variant)
    return _nc_cache[key]


HONEST = True
VARIANT = "full"


def kernel(q, k, v, k_cache, v_cache, slot_mapping, cu_seqlens, _trace=False):
    q = np.asarray(q, dtype=np.float32)
    k = np.asarray(k, dtype=np.float32)
    v = np.asarray(v, dtype=np.float32)
    slot_mapping = np.asarray(slot_mapping, dtype=np.int32)
    cu_seqlens = np.asarray(cu_seqlens, dtype=np.int32)

    if not _inputs_match_specialization(
        q, k, v, k_cache, v_cache, slot_mapping, cu_seqlens
    ):
        return _numpy_reference(
            q, k, v, k_cache, v_cache, slot_mapping, cu_seqlens
        )

    nc = _get_nc(HONEST, VARIANT)
    in_maps = _shard_inputs(q, k, v, slot_mapping)
    res = run_bass_kernel_spmd(
        nc, in_maps, core_ids=list(range(N_CORES)), trace=_trace
    )
    out = _assemble(res.results)
    if _trace:
        kernel._last_bench = res
    return out


# revision 24
# speedup vs baseline: 1.2267x; 1.0141x over previous
"""Distributed Trainium2 kernel for varlen GQA prefill attention with a
paged-KV-cache scatter (vLLM-style store_kvcache + flash_attn_varlen).

Sharding (8 NeuronCores): tensor-parallel over the 4 KV heads (4 groups
x 4 query heads each) x data-parallel over the 2 token halves (the 4
sequences of 512 tokens split 2/2). Each core's output slice is
disjoint, so no collectives are needed; the KV-cache scatter/gather is
replicated per shard on that shard's kv-head slice.
"""

import sys

for _p in ("/opt/trn_rl_repo", "/opt/trn_rl_repo/concourse"):
    if _p not in sys.path:
        sys.path.insert(0, _p)

import math

import ml_dtypes
import numpy as np

import concourse.bass as bass
import concourse.mybir as mybir
import concourse.tile as tile
from concourse import bacc
from concourse.bass import ds, ts
from concourse.bass_utils import run_bass_kernel_spmd
from concourse.masks import make_identity

BF16 = ml_dtypes.bfloat16

N = 2048
HQ = 16
HKV = 4
D = 128
NUM_SLOTS = 131072
SEQ = 512
SCALE = 1.0 / math.sqrt(D)

P = 128
N_CORES = 8
TOK = N // 2          # tokens per core (two halves)
NSEG = TOK // SEQ     # segments per core (2)
NH = HQ // HKV        # q heads per core (4)
NT = TOK // P         # 128-token tiles per core (8)
NKT = SEQ // P        # 128-token tiles per segment (4)

_nc_cache = {}


def build(honest: bool, variant: str = "full"):
    nc = bacc.Bacc(None, target_bir_lowering=False)
    f32 = mybir.dt.float32
    bf16 = mybir.dt.bfloat16
    i32 = mybir.dt.int32

    qT_in = nc.declare_dram_parameter("qT", [P, NH, TOK], bf16, isOutput=False)
    tri_in = nc.declare_dram_parameter("tri", [P, NKT, P], bf16, isOutput=False)
    if honest:
        kvR_in = nc.declare_dram_parameter("kvR", [P, NT, 2 * D], bf16, isOutput=False)
        sl_in = nc.declare_dram_parameter("slots", [P, NT], i32, isOutput=False)
    if (not honest) or variant == "vA_rhs":
        kT_in = nc.declare_dram_parameter("kT", [P, TOK], bf16, isOutput=False)
        vA_in = nc.declare_dram_parameter("vA", [P, NT, D + 1], bf16, isOutput=False)
    o_out = nc.declare_dram_parameter("o", [P, NH, NT, D], bf16, isOutput=True)

    with tile.TileContext(nc) as tc:
        with (
            tc.tile_pool(name="persist", bufs=1) as pp,
            tc.tile_pool(name="sc_psum", bufs=2, space="PSUM") as scp,
            tc.tile_pool(name="pv_psum", bufs=3, space="PSUM") as pvp,
            tc.tile_pool(name="warm_psum", bufs=1, space="PSUM") as wmp,
            tc.tile_pool(name="work", bufs=3) as wp,
            tc.tile_pool(name="small", bufs=4) as sp,
        ):
            qT_sb = pp.tile([P, NH, TOK], bf16, tag="qT_sb")
            tri_sb = pp.tile([P, NKT, P], bf16, tag="tri_sb")
            kT_sb = pp.tile([P, TOK], bf16, tag="kT_sb")
            vA_sb = pp.tile([P, NT, D + 1], bf16, tag="vA_sb")
            o_sb = pp.tile([P, NH, NT, D], bf16, tag="o_sb")

            # Warm up the PE HAM clock-gate while the input DMAs land:
            # dummy matmuls on a scratch tile keep TensorE busy >3.4us so
            # the real matmuls run at 2.4GHz from the start.
            junk_sb = pp.tile([P, SEQ], bf16, tag="junk_sb")
            junk_ps = wmp.tile([P, SEQ], f32, tag="junk_ps")
            nc.gpsimd.memset(junk_sb[:], 0.125)
            for _ in range(7):
                nc.tensor.matmul(
                    junk_ps[:], lhsT=junk_sb[:, 0:P], rhs=junk_sb[:],
                    start=True, stop=True,
                )

            if not honest:
                nc.sync.dma_start(out=kT_sb[:], in_=kT_in[:])
            for h in range(NH):
                nc.sync.dma_start(out=qT_sb[:, h, :], in_=qT_in[:, h, :])
            if (not honest) or variant == "vA_rhs":
                nc.sync.dma_start(out=vA_sb[:], in_=vA_in[:])
            nc.sync.dma_start(out=tri_sb[:], in_=tri_in[:])

            if honest and variant != "attn_only":
                with tc.tile_pool(name="tables", bufs=1, space="DRAM") as dp, \
                     tc.tile_pool(name="tp_psum", bufs=1, space="PSUM") as tpp:
                    # one private [NUM_SLOTS, 256] kv table per 128-token
                    # tile so the 8 scatter->gather pairs stay independent
                    tables = [
                        dp.tile([NUM_SLOTS, 2 * D], bf16, name=f"kv_table{c}", tag=f"kv_table{c}")
                        for c in range(NT)
                    ]
                    kvR_sb = pp.tile([P, NT, 2 * D], bf16, tag="kvR_sb")
                    kvG_sb = pp.tile([P, NT, 2 * D + 2], bf16, tag="kvG_sb")
                    sl_sb = pp.tile([P, NT], i32, tag="sl_sb")
                    ident = pp.tile([P, P], bf16, tag="ident")
                    make_identity(nc, ident[:])

                    nc.sync.dma_start(out=sl_sb[:], in_=sl_in[:])
                    nc.sync.dma_start(out=kvR_sb[:], in_=kvR_in[:])
                    nc.vector.memset(kvG_sb[:, :, 2 * D : 2 * D + 1], 1.0)

                    for c in range(NT):
                        # scatter the 128 [k|v] rows of tile c, read them
                        # back (the paged-read), transpose K for the QK^T
                        nc.gpsimd.indirect_dma_start(
                            out=tables[c][:],
                            out_offset=bass.IndirectOffsetOnAxis(
                                ap=sl_sb[:, c : c + 1], axis=0
                            ),
                            in_=kvR_sb[:, c, :],
                            in_offset=None,
                        )
                        nc.gpsimd.indirect_dma_start(
                            out=kvG_sb[:, c, 0 : 2 * D],
                            out_offset=None,
                            in_=tables[c][:],
                            in_offset=bass.IndirectOffsetOnAxis(
                                ap=sl_sb[:, c : c + 1], axis=0
                            ),
                        )
                        tp = tpp.tile([P, P], bf16, tag="tp")
                        nc.tensor.transpose(tp[:], kvG_sb[:, c, 0:D], ident[:])
                        nc.vector.tensor_copy(out=kT_sb[:, ts(c, P)], in_=tp[:])
            if variant == "scatter_only":
                nc.vector.memset(o_sb[:], 0.0)
                nc.sync.dma_start(out=o_out[:], in_=o_sb[:])
            for seg in range(NSEG if variant != "scatter_only" else 0):
                for h in range(NH):
                    expT = wp.tile([P, NKT, SEQ], bf16, tag="expT")
                    for kp in range(NKT // 2):
                        kt0 = 2 * kp
                        n_q0 = SEQ - kt0 * P
                        n_q1 = SEQ - (kt0 + 1) * P
                        sc = scp.tile([P, 2, SEQ], f32, tag="sc")
                        for j, (kt, n_q) in enumerate(
                            ((kt0, n_q0), (kt0 + 1, n_q1))
                        ):
                            q0 = seg * SEQ + kt * P
                            nc.tensor.matmul(
                                sc[:, j, :n_q],
                                lhsT=kT_sb[:, ds(seg * SEQ + kt * P, P)],
                                rhs=qT_sb[:, h, ds(q0, n_q)],
                                start=True,
                                stop=True,
                            )
                        # one exp over both banks; the tail of block kt0+1
                        # past n_q1 is never-read scratch
                        nc.scalar.activation(
                            expT[:, kt0 : kt0 + 2, :n_q0],
                            sc[:, :, :n_q0],
                            mybir.ActivationFunctionType.Exp,
                            scale=SCALE,
                        )
                        # mask the (upper-triangular in [k,q]) diagonal
                        # blocks on the otherwise-idle GpSimd engine
                        nc.gpsimd.tensor_tensor(
                            out=expT[:, kt0 : kt0 + 2, 0:P],
                            in0=expT[:, kt0 : kt0 + 2, 0:P],
                            in1=tri_sb[:, kt0 : kt0 + 2, :],
                            op=mybir.AluOpType.mult,
                        )
                    for qp in range(NKT // 2):
                        pv = pvp.tile([P, 2, D + 1], f32, tag="pv")
                        for j in range(2):
                            qt = 2 * qp + j
                            for kt in range(qt + 1):
                                c = seg * NKT + kt
                                if honest and variant != "vA_rhs":
                                    rhs = kvG_sb[:, c, D : 2 * D + 1]
                                else:
                                    rhs = vA_sb[:, c, :]
                                nc.tensor.matmul(
                                    pv[:, j, :],
                                    lhsT=expT[:, kt, ds((qt - kt) * P, P)],
                                    rhs=rhs,
                                    start=(kt == 0),
                                    stop=(kt == qt),
                                )
                        rec = sp.tile([P, 2], f32, tag="rec")
                        nc.vector.reciprocal(rec[:], pv[:, :, D])
                        nc.vector.tensor_tensor(
                            out=o_sb[:, h, ds(seg * NKT + 2 * qp, 2), :],
                            in0=pv[:, :, 0:D],
                            in1=rec[:, :, None].to_broadcast([P, 2, D]),
                            op=mybir.AluOpType.mult,
                        )
                        nc.sync.dma_start(
                            out=o_out[:, h, ds(seg * NKT + 2 * qp, 2), :],
                            in_=o_sb[:, h, ds(seg * NKT + 2 * qp, 2), :],
                        )
    nc.compile()
    return nc


def _shard_inputs(q, k, v, slot_mapping):
    tri = (np.arange(P)[:, None] <= np.arange(P)[None, :]).astype(BF16)
    tri = np.ascontiguousarray(np.broadcast_to(tri[:, None, :], (P, NKT, P)))
    in_maps = []
    for c in range(N_CORES):
        hg, tg = c // 2, c % 2
        t0 = tg * TOK
        q_sh = q[t0 : t0 + TOK, hg * NH : (hg + 1) * NH, :]
        qT = np.ascontiguousarray(q_sh.transpose(2, 1, 0)).astype(BF16)
        k_sh = k[t0 : t0 + TOK, hg, :]
        v_sh = v[t0 : t0 + TOK, hg, :]
        kvR = np.empty((P, NT, 2 * D), dtype=BF16)
        kvR[:, :, :D] = k_sh.reshape(NT, P, P).transpose(1, 0, 2)
        kvR[:, :, D:] = v_sh.reshape(NT, P, P).transpose(1, 0, 2)
        kT = np.ascontiguousarray(k_sh.T).astype(BF16)
        vA = np.empty((P, NT, D + 1), dtype=BF16)
        vA[:, :, :D] = kvR[:, :, D:]
        vA[:, :, D] = 1.0
        slots = np.ascontiguousarray(
            slot_mapping[t0 : t0 + TOK].reshape(NT, P).T
        ).astype(np.int32)
        in_maps.append(
            {
                "qT": qT,
                "tri": tri,
                "kT": kT,
                "vA": vA,
                "kvR": kvR,
                "slots": slots,
            }
        )
    return in_maps


def _assemble(results):
    out = np.empty((N, HQ, D), dtype=np.float32)
    for c in range(N_CORES):
        hg, tg = c // 2, c % 2
        t0 = tg * TOK
        oc = np.asarray(results[c]["o"]).astype(np.float32)  # [P, NH, NT, D]
        # token t0 + ct*128 + p, head hg*NH + h  <-  oc[p, h, ct, :]
        out[t0 : t0 + TOK, hg * NH : (hg + 1) * NH, :] = oc.transpose(
            2, 0, 1, 3
        ).reshape(TOK, NH, D)
    return out


def _numpy_reference(q, k, v, k_cache, v_cache, slot_mapping, cu_seqlens):
    """Bit-faithful numpy fallback used only if inputs don't match the
    shapes/metadata this kernel was specialized for."""
    n = q.shape[0]
    k_cache = np.array(k_cache, dtype=np.float32, copy=True)
    v_cache = np.array(v_cache, dtype=np.float32, copy=True)
    sm = slot_mapping.astype(np.int64)
    valid = sm >= 0
    k_cache[sm[valid]] = k.reshape(n, -1)[valid]
    v_cache[sm[valid]] = v.reshape(n, -1)[valid]
    read = np.clip(sm, 0, k_cache.shape[0] - 1)
    kc = k_cache[read].reshape(n, HKV, D)
    vc = v_cache[read].reshape(n, HKV, D)
    pos = np.arange(n)
    seg = np.searchsorted(cu_seqlens, pos, side="right") - 1
    group = q.shape[1] // kc.shape[1]
    ke = np.repeat(kc, group, axis=1)
    ve = np.repeat(vc, group, axis=1)
    scores = np.einsum("qhd,khd->hqk", q, ke, dtype=np.float32) * np.float32(SCALE)
    mask = (seg[:, None] == seg[None, :]) & (pos[None, :] <= pos[:, None])
    scores = np.where(mask[None], scores, -np.inf)
    scores -= scores.max(axis=-1, keepdims=True)
    p = np.exp(scores)
    p /= p.sum(axis=-1, keepdims=True)
    return np.einsum("hqk,khd->qhd", p, ve).astype(np.float32)


def _inputs_match_specialization(q, k, v, k_cache, v_cache, slot_mapping, cu_seqlens):
    if q.shape != (N, HQ, D) or k.shape != (N, HKV, D) or v.shape != (N, HKV, D):
        return False
    if k_cache.shape != (NUM_SLOTS, HKV * D) or v_cache.shape != (NUM_SLOTS, HKV * D):
        return False
    if not np.array_equal(cu_seqlens, np.arange(0, N + 1, SEQ)):
        return False
    sm = np.asarray(slot_mapping)
    if sm.shape != (N,):
        return False
    if sm.min() < 0 or sm.max() >= NUM_SLOTS:
        return False
    if np.unique(sm).size != N:
        return False
    # kernel assumes the caches start zeroed only insofar as unwritten
    # slots are never read back, which holds when all slots are distinct
    return True


def _get_nc(honest: bool, variant: str = "full"):
    key = ("honest" if honest else "skip", variant)
    if key not in _nc_cache:
        _nc_cache[key] = build(honest, variant)
    return _nc_cache[key]


HONEST = True
VARIANT = "full"


def kernel(q, k, v, k_cache, v_cache, slot_mapping, cu_seqlens, _trace=False):
    q = np.asarray(q, dtype=np.float32)
    k = np.asarray(k, dtype=np.float32)
    v = np.asarray(v, dtype=np.float32)
    slot_mapping = np.asarray(slot_mapping, dtype=np.int32)
    cu_seqlens = np.asarray(cu_seqlens, dtype=np.int32)

    if not _inputs_match_specialization(
        q, k, v, k_cache, v_cache, slot_mapping, cu_seqlens
    ):
        return _numpy_reference(
            q, k, v, k_cache, v_cache, slot_mapping, cu_seqlens
        )

    nc = _get_nc(HONEST, VARIANT)
    in_maps = _shard_inputs(q, k, v, slot_mapping)
    res = run_bass_kernel_spmd(
        nc, in_maps, core_ids=list(range(N_CORES)), trace=_trace
    )
    out = _assemble(res.results)
    if _trace:
        kernel._last_bench = res
    return out
